# revision 24
# baseline (speedup 1.0000x reference)
"""Trainium2 Bass kernel for nn_AttentionBlock (B=4, N=1024, C=1024, H=16).

Sharding: 8 cores = 4 batches x 2 head-groups (8 heads each). Each core
computes its batch's tokens for its 8 heads end-to-end; the host sums the
two partial output projections per batch.

v2 design notes (vs the fp32r baseline):
- All matmul operands are bf16 (PSUM accumulation stays fp32). Inputs are
  converted host-side; DMA traffic and DVE element counts halve.
- Loop nests keep TensorE dense so the HAM clock-gate stays at 2.4 GHz.
- qk-LayerNorm stats via one bn_stats pass + per-head bn_aggr; rstd is
  exp(-0.5*ln(var+eps)) so ScalarE only ever uses the natural_log_exp
  table set (no ACT table reloads; softmax exp shares the same set).
- v tiles carry 64 ones-columns per head, so the attention a@v matmul
  also produces the softmax denominator replicated across the unused 64
  output partitions. Even heads store [v|ones], odd heads [ones|v], which
  lane-aligns values with outT rows for the normalize multiply.
- Softmax normalize: DVE reciprocal of one denominator row + GpSimd
  partition_broadcast + DVE multiply (no DRAM round-trip).
- exp is batched [128, 2x512] across the two heads of a pair (one ACT op
  per key tile).
"""
import os
import sys

sys.path.insert(0, "/opt/trn_rl_repo")

import numpy as np
import ml_dtypes

import concourse.bass as bass
import concourse.bacc as bacc
import concourse.tile as tile
from concourse import mybir
from concourse.bass_utils import run_bass_kernel_spmd
from concourse.masks import make_identity

F32 = mybir.dt.float32
BF16 = mybir.dt.bfloat16
NPBF = ml_dtypes.bfloat16

B, N, C, H = 4, 1024, 1024, 16
DH = C // H            # 64
HPC = 8                # heads per core
NT = N // 128          # 8 token tiles
KC = (2 * C) // 128    # 16 contraction chunks for fused qkv+dt
HD = DH // 2           # 32 (rope half)
EPS = 1e-5
ALU = mybir.AluOpType
AF = mybir.ActivationFunctionType
AXX = mybir.AxisListType.X


def _bcast_free(ap, n, axis_pos=1):
    """Insert a step-0 free dim of size n at axis_pos of an AP."""
    new = list(ap.ap)
    new.insert(axis_pos, [0, n])
    return bass.AP(tensor=ap.tensor, offset=ap.offset, ap=new)


def _bcast_part(ap, n):
    """Partition-broadcast AP (step-0 partition dim) for DMA use."""
    return bass.AP(tensor=ap.tensor, offset=ap.offset,
                   ap=[[0, n]] + list(ap.ap[1:]))


def build(l1):
    """Single-core SPMD program. l1: python float (lamb1). lamb2 is folded
    into vres host-side. Zero qkv/dt biases and identity qk-LN assumed
    (verified by kernel(); otherwise a numpy fallback runs)."""
    nc = bacc.Bacc("TRN2", target_bir_lowering=False)

    # All inputs pre-arranged host-side to [128, X] partition-major
    # contiguous layouts so each is one full-bandwidth DMA.
    xdT = nc.dram_tensor("xdT", [128, KC * N], BF16, kind="ExternalInput")
    w = nc.dram_tensor("w", [128, 3 * KC * 512], BF16, kind="ExternalInput")
    vres = nc.dram_tensor("vres", [128, NT * 512], BF16,
                          kind="ExternalInput")
    wproj = nc.dram_tensor("wproj", [128, 4 * C], BF16, kind="ExternalInput")
    rope = nc.dram_tensor("rope", [128, NT * 2 * DH], BF16,
                          kind="ExternalInput")
    outd = nc.dram_tensor("out", [N, C], BF16, kind="ExternalOutput")

    with tile.TileContext(nc) as tc:
        with (
            tc.tile_pool(name="const", bufs=1) as constp,
            tc.tile_pool(name="longp", bufs=1) as longp,
        ):
            ident = constp.tile([128, 128], BF16)
            make_identity(nc, ident)
            eps_t = constp.tile([128, 1], F32)
            nc.vector.memset(eps_t, EPS)

            xdT_sb = longp.tile([128, NT, KC, 128], BF16)
            w_sb = longp.tile([128, 3, KC, 512], BF16)
            rp_sb = longp.tile([128, NT, 2 * DH], BF16)
            vres_sb = longp.tile([128, NT, 512], BF16)
            wproj_sb = longp.tile([128, 4, C], BF16)
            v_sb = longp.tile([128, NT, HPC, 128], BF16)
            qT_sb = longp.tile([128, 4, N], BF16)
            kT_sb = longp.tile([128, 4, N], BF16)
            outT_sb = longp.tile([128, 4, N], BF16)
            # staging for the two-pass LN: raw q/k projections; the rope
            # output is written back IN PLACE (the raw values are dead by
            # then), so this doubles as the pre-transpose q/k buffer.
            stage_sb = longp.tile([128, 2, NT, 512], BF16)
            sums_sb = longp.tile([128, 2, NT, HPC], F32)
            sqs_sb = longp.tile([128, 2, NT, HPC], F32)
            rstd_sb = longp.tile([128, 2, NT, HPC], F32)
            nmr_sb = longp.tile([128, 2, NT, HPC], F32)

            # ones columns in v tiles: a@v against them replicates the
            # softmax denominator on the same lanes in a second PSUM bank.
            nc.gpsimd.memset(v_sb[:, :, :, 64:128], 1.0)

            # ---- input DMAs (contiguous, ordered by first use) ----
            nc.sync.dma_start(out=rp_sb, in_=rope[:, :])
            for c4 in range(4):  # k weights first, 4-kc chunks
                nc.sync.dma_start(
                    out=w_sb[:, 1, 4 * c4:4 * c4 + 4, :],
                    in_=w[:, (KC + 4 * c4) * 512:(KC + 4 * c4 + 4) * 512])
            for t in range(NT):  # x/dt in token-tile-major chunks
                nc.sync.dma_start(
                    out=xdT_sb[:, t, :, :],
                    in_=xdT[:, t * KC * 128:(t + 1) * KC * 128])
            nc.sync.dma_start(out=w_sb[:, 0, :, :], in_=w[:, 0:KC * 512])
            nc.sync.dma_start(out=w_sb[:, 2, :, :],
                              in_=w[:, 2 * KC * 512:3 * KC * 512])
            nc.sync.dma_start(out=vres_sb, in_=vres[:, :])
            nc.sync.dma_start(out=wproj_sb, in_=wproj[:, :])

            with (
                tc.tile_pool(name="psA", bufs=2, space="PSUM") as psA,
                tc.tile_pool(name="scr", bufs=2) as scr,
                tc.tile_pool(name="sml", bufs=2) as sml,
                tc.tile_pool(name="psS", bufs=2, space="PSUM") as psS,
                tc.tile_pool(name="psV", bufs=2, space="PSUM") as psV,
                tc.tile_pool(name="expp", bufs=11) as expp,
                tc.tile_pool(name="nrmp", bufs=2) as nrmp,
                tc.tile_pool(name="outp", bufs=2) as outp,
            ):
                def mm_tile(ob, t):
                    """16 accumulating matmuls for one (ob, token-tile)."""
                    ps = psA.tile([128, 512], F32, tag="ps")
                    for kc in range(KC):
                        nc.tensor.matmul(
                            ps[:],
                            xdT_sb[:, t, kc, :],
                            w_sb[:, ob, kc, :],
                            start=(kc == 0), stop=(kc == KC - 1))
                    return ps

                def a1_post(ob, t, ps):
                    """Stage the raw projection + accumulate LN stats."""
                    st3 = stage_sb[:, ob, t, :].rearrange(
                        "p (h d) -> p h d", h=HPC)
                    nc.scalar.copy(out=st3,
                                   in_=ps.rearrange("p (h d) -> p h d",
                                                    h=HPC))
                    nc.vector.reduce_sum(out=sums_sb[:, ob, t, :], in_=st3,
                                         axis=AXX)
                    sqt = scr.tile([128, HPC, DH], BF16, tag="sqt")
                    nc.gpsimd.tensor_tensor(out=sqt[:], in0=st3, in1=st3,
                                            op=ALU.mult)
                    nc.vector.reduce_sum(out=sqs_sb[:, ob, t, :], in_=sqt[:],
                                         axis=AXX)

                def stats_batch(ob, ts=slice(0, NT)):
                    """rstd/-mean*rstd for a token-tile range of one ob in
                    two ACT ops (avoids per-tile Ln/Exp table thrash)."""
                    nts = ts.stop - ts.start
                    mean = sml.tile([128, NT, HPC], F32, tag="mean")
                    mean = mean[:, 0:nts, :]
                    nc.vector.tensor_scalar_mul(mean,
                                                in0=sums_sb[:, ob, ts, :],
                                                scalar1=1.0 / DH)
                    msq = sml.tile([128, NT, HPC], F32, tag="msq")
                    msq = msq[:, 0:nts, :]
                    nc.vector.tensor_tensor(out=msq, in0=mean,
                                            in1=mean, op=ALU.mult)
                    var = sml.tile([128, NT, HPC], F32, tag="var")
                    var = var[:, 0:nts, :]
                    nc.vector.scalar_tensor_tensor(
                        out=var, in0=sqs_sb[:, ob, ts, :], scalar=1.0 / DH,
                        in1=msq, op0=ALU.mult, op1=ALU.subtract)
                    lv = sml.tile([128, NT, HPC], F32, tag="lv")
                    lv = lv[:, 0:nts, :]
                    nc.scalar.activation(out=lv, in_=var, func=AF.Ln,
                                         bias=eps_t[:])
                    nc.scalar.activation(out=rstd_sb[:, ob, ts, :], in_=lv,
                                         func=AF.Exp, scale=-0.5)
                    nc.vector.scalar_tensor_tensor(
                        out=nmr_sb[:, ob, ts, :], in0=mean, scalar=-1.0,
                        in1=rstd_sb[:, ob, ts, :], op0=ALU.mult, op1=ALU.mult)

                def a2_tile(ob, t):
                    """LN apply + rope, writing back into stage_sb."""
                    st3 = stage_sb[:, ob, t, :].rearrange(
                        "p (h d) -> p h d", h=HPC)
                    tmp = scr.tile([128, HPC, DH], BF16, tag="tmp")
                    nc.vector.tensor_tensor(
                        out=tmp[:], in0=st3,
                        in1=_bcast_free(rstd_sb[:, ob, t, :], DH, 2),
                        op=ALU.mult)
                    ln = scr.tile([128, HPC, DH], BF16, tag="ln")
                    nc.vector.tensor_tensor(
                        out=ln[:], in0=tmp[:],
                        in1=_bcast_free(nmr_sb[:, ob, t, :], DH, 2),
                        op=ALU.add)
                    sin0 = _bcast_free(rp_sb[:, t, 0:HD], HPC, 1)
                    sin1 = _bcast_free(rp_sb[:, t, HD:DH], HPC, 1)
                    cos0 = _bcast_free(rp_sb[:, t, DH:DH + HD], HPC, 1)
                    cos1 = _bcast_free(rp_sb[:, t, DH + HD:2 * DH], HPC, 1)
                    t1 = scr.tile([128, HPC, HD], BF16, tag="t1")
                    t2 = scr.tile([128, HPC, HD], BF16, tag="t2")
                    t3 = scr.tile([128, HPC, HD], BF16, tag="t3")
                    t4 = scr.tile([128, HPC, HD], BF16, tag="t4")
                    lo = ln[:, :, 0:HD]
                    hi = ln[:, :, HD:DH]
                    nc.gpsimd.tensor_tensor(out=t1[:], in0=hi, in1=sin0[:],
                                            op=ALU.mult)
                    nc.vector.tensor_tensor(out=t2[:], in0=lo, in1=cos0[:],
                                            op=ALU.mult)
                    nc.vector.tensor_tensor(out=st3[:, :, 0:HD],
                                            in0=t2[:], in1=t1[:],
                                            op=ALU.subtract)
                    nc.gpsimd.tensor_tensor(out=t3[:], in0=lo, in1=sin1[:],
                                            op=ALU.mult)
                    nc.vector.tensor_tensor(out=t4[:], in0=hi, in1=cos1[:],
                                            op=ALU.mult)
                    nc.vector.tensor_tensor(out=st3[:, :, HD:DH],
                                            in0=t4[:], in1=t3[:],
                                            op=ALU.add)

                def transposes(ob, dstT, th):
                    for j in range(4):
                        pt = psS.tile([128, 512], BF16, tag="sc")
                        for i in range(4):
                            t = th * 4 + i
                            nc.tensor.transpose(
                                pt[:, i * 128:(i + 1) * 128],
                                stage_sb[:, ob, t,
                                         j * 128:(j + 1) * 128],
                                ident[:])
                        nc.scalar.copy(
                            out=dstT[:, j, th * 512:(th + 1) * 512],
                            in_=pt[:])

                def sc_kc(j, qh, kc, exs):
                    """One score pair + exp for (j, qh) at key tile kc."""
                    qsl = slice(qh * 512, (qh + 1) * 512)
                    ksl = slice(kc * 128, (kc + 1) * 128)
                    sct = psS.tile([128, 2, 512], F32, tag="sc")
                    nc.tensor.matmul(
                        sct[:, 0, :], kT_sb[0:64, j, ksl],
                        qT_sb[0:64, j, qsl],
                        start=True, stop=True, tile_position=(0, 0))
                    nc.tensor.matmul(
                        sct[:, 1, :], kT_sb[64:128, j, ksl],
                        qT_sb[64:128, j, qsl],
                        start=True, stop=True, tile_position=(64, 0))
                    ex = expp.tile([128, 2, 512], BF16, tag="ex")
                    nc.scalar.activation(out=ex[:], in_=sct[:],
                                         func=AF.Exp, scale=0.125)
                    exs.append(ex)

                def sc_exp(j, qh, exs):
                    for kc in range(NT):
                        sc_kc(j, qh, kc, exs)

                def av_kc(st8, kc):
                    """a@v + denominator matmuls for one key tile of the
                    LAGGING head pair (software pipeline stage 2)."""
                    j, qh, exs, avv, avd = st8
                    ex = exs[kc]
                    st = (kc == 0)
                    sp = (kc == NT - 1)
                    nc.tensor.matmul(
                        avv[0:64, :], v_sb[:, kc, 2 * j, 0:64],
                        ex[:, 0, :], start=st, stop=sp,
                        tile_position=(0, 0))
                    nc.tensor.matmul(
                        avd[0:64, :], v_sb[:, kc, 2 * j, 64:128],
                        ex[:, 0, :], start=st, stop=sp,
                        tile_position=(0, 0))
                    nc.tensor.matmul(
                        avv[64:128, :], v_sb[:, kc, 2 * j + 1, 0:64],
                        ex[:, 1, :], start=st, stop=sp,
                        tile_position=(0, 64))
                    nc.tensor.matmul(
                        avd[64:128, :], v_sb[:, kc, 2 * j + 1, 64:128],
                        ex[:, 1, :], start=st, stop=sp,
                        tile_position=(0, 64))

                def norm(st8):
                    """Normalize the lagging pair: copy to SBUF (frees the
                    PSUM banks), reciprocal + multiply on DVE."""
                    j, qh, exs, avv, avd = st8
                    qsl = slice(qh * 512, (qh + 1) * 512)
                    exs.clear()
                    va = nrmp.tile([128, 512], F32, tag="va")
                    nc.vector.tensor_copy(va[:], avv[:])
                    da = nrmp.tile([128, 512], F32, tag="da")
                    nc.vector.tensor_copy(da[:], avd[:])
                    rcb = nrmp.tile([128, 512], F32, tag="rcb")
                    nc.vector.reciprocal(rcb[:], da[:])
                    nc.vector.tensor_tensor(
                        out=outT_sb[:, j, qsl], in0=va[:], in1=rcb[:],
                        op=ALU.mult)

                def proj_tile(t):
                    ost = outp.tile([128, C], BF16, tag="ost")
                    for oh in range(2):
                        pp = psA.tile([128, 512], F32, tag="ps")
                        for cc in range(4):
                            nc.tensor.matmul(
                                pp[:],
                                outT_sb[:, cc, t * 128:(t + 1) * 128],
                                wproj_sb[:, cc, oh * 512:(oh + 1) * 512],
                                start=(cc == 0), stop=(cc == 3))
                        nc.vector.tensor_copy(
                            ost[:, oh * 512:(oh + 1) * 512], pp[:])
                    nc.sync.dma_start(
                        out=outd[t * 128:(t + 1) * 128, :], in_=ost[:])

                def v_tile(t):
                    ps = mm_tile(2, t)
                    ps3 = ps.rearrange("p (h d) -> p h d", h=HPC)
                    vt3 = vres_sb[:, t, :].rearrange("p (h d) -> p h d",
                                                     h=HPC)
                    nc.vector.scalar_tensor_tensor(
                        out=v_sb[:, t, :, 0:64], in0=ps3, scalar=l1,
                        in1=vt3, op0=ALU.mult, op1=ALU.add)

                # ---- phase A (ordered to start the exp stream early) ----
                for t in range(NT):
                    a1_post(1, t, mm_tile(1, t))        # k
                stats_batch(1)
                for t in range(4):                      # v tiles 0-3
                    v_tile(t)
                    a2_tile(1, t)                       # k LN+rope rides
                for t in range(NT):                     # q
                    a1_post(0, t, mm_tile(0, t))
                    if t < 4:
                        a2_tile(1, t + 4)
                    else:
                        if t == 4:
                            stats_batch(0, slice(0, 4))
                        a2_tile(0, t - 4)
                transposes(1, kT_sb, 0)
                transposes(1, kT_sb, 1)
                transposes(0, qT_sb, 0)
                exs0 = []
                sc_exp(0, 0, exs0)                      # exp stream starts
                stats_batch(0, slice(4, NT))
                for t in range(4, NT):                  # v tiles 4-7
                    v_tile(t)
                    a2_tile(0, t)
                transposes(0, qT_sb, 1)

                # ---- attention + projection (av lags one pair) ----
                avv0 = psV.tile([128, 512], F32, tag="av")
                avd0 = psV.tile([128, 512], F32, tag="av")
                prev = (0, 0, exs0, avv0, avd0)
                projq = []
                seq = [(j, qh) for qh in range(2) for j in range(4)]
                for j, qh in seq[1:]:
                    exs = []
                    for kc in range(NT):
                        sc_kc(j, qh, kc, exs)
                        av_kc(prev, kc)
                    norm(prev)
                    if prev[0] == 3 and prev[1] == 0:
                        projq = [0, 1, 2, 3]
                    if projq:
                        proj_tile(projq.pop(0))
                    avvn = psV.tile([128, 512], F32, name=f"avv{j}{qh}",
                                    tag="av")
                    avdn = psV.tile([128, 512], F32, name=f"avd{j}{qh}",
                                    tag="av")
                    prev = (j, qh, exs, avvn, avdn)
                for kc in range(NT):
                    av_kc(prev, kc)
                norm(prev)
                for t in projq + [4, 5, 6, 7]:
                    proj_tile(t)

    nc.finalize()
    return nc


def _np_reference(x, rope, delta_t_emb, v_residual_v1, Wqkv, bqkv, Wdt, bdt,
                  qn_g, qn_b, kn_g, kn_b, lamb1, lamb2, Wproj, bproj):
    """Plain numpy fallback for input regimes the device kernel doesn't
    specialize (nonzero qkv/dt bias or non-identity qk-LN affine)."""
    b, n, c = x.shape
    qkv = (x @ Wqkv.T + bqkv).reshape(b, n, 3, H, DH).transpose(2, 0, 3, 1, 4)
    dqkv = (delta_t_emb @ Wdt.T + bdt).reshape(b, n, 3, H, DH)
    dqkv = dqkv.transpose(2, 0, 3, 1, 4)
    q = qkv[0] + dqkv[0]
    k = qkv[1] + dqkv[1]
    v = lamb1 * (qkv[2] + dqkv[2]) + lamb2 * v_residual_v1

    def ln(t, g, bb):
        m = t.mean(-1, keepdims=True)
        va = ((t - m) ** 2).mean(-1, keepdims=True)
        return (t - m) / np.sqrt(va + EPS) * g + bb

    q = ln(q, qn_g, qn_b)
    k = ln(k, kn_g, kn_b)
    sin, cos = rope[:, :DH], rope[:, DH:]

    def rot(t):
        h2 = np.concatenate([-t[..., DH // 2:], t[..., :DH // 2]], -1)
        return t * cos + h2 * sin

    q = rot(q)
    k = rot(k)
    s = np.einsum('bhqd,bhkd->bhqk', q, k) / np.sqrt(DH)
    s = s - s.max(-1, keepdims=True)
    e = np.exp(s)
    a = e / e.sum(-1, keepdims=True)
    o = np.einsum('bhqk,bhkd->bhqd', a, v)
    o = o.transpose(0, 2, 1, 3).reshape(b, n, c)
    return o @ Wproj.T + bproj


_CACHE = {}


def kernel(x, rope, delta_t_emb, v_residual_v1, Wqkv, bqkv, Wdt, bdt,
           qn_g, qn_b, kn_g, kn_b, lamb1, lamb2, Wproj, bproj):
    x = np.asarray(x, np.float32)
    rope = np.asarray(rope, np.float32)
    delta_t_emb = np.asarray(delta_t_emb, np.float32)
    v_residual_v1 = np.asarray(v_residual_v1, np.float32)
    Wqkv = np.asarray(Wqkv, np.float32)
    Wdt = np.asarray(Wdt, np.float32)
    Wproj = np.asarray(Wproj, np.float32)
    bqkv = np.asarray(bqkv, np.float32)
    bdt = np.asarray(bdt, np.float32)
    bproj = np.asarray(bproj, np.float32)
    qn_g = np.asarray(qn_g, np.float32)
    qn_b = np.asarray(qn_b, np.float32)
    kn_g = np.asarray(kn_g, np.float32)
    kn_b = np.asarray(kn_b, np.float32)
    l1 = float(np.asarray(lamb1))
    l2 = float(np.asarray(lamb2))

    general = (np.any(bqkv) or np.any(bdt) or np.any(qn_g != 1.0)
               or np.any(qn_b) or np.any(kn_g != 1.0) or np.any(kn_b))
    if general:
        return _np_reference(
            x, rope, delta_t_emb, v_residual_v1, Wqkv, bqkv, Wdt, bdt,
            qn_g, qn_b, kn_g, kn_b, l1, l2, Wproj, bproj).astype(np.float32)

    if l1 not in _CACHE:
        _CACHE[l1] = build(l1)
    nc = _CACHE[l1]

    def pmaj(a):
        """[G*128, X] -> [128, G*X] partition-major contiguous bf16."""
        g = a.shape[0] // 128
        return np.ascontiguousarray(
            a.reshape(g, 128, -1).transpose(1, 0, 2).reshape(128, -1)
        ).astype(NPBF)

    rope_r = pmaj(rope)
    in_maps = []
    for core in range(8):
        b = core // 2
        g = core % 2
        rsl = slice(g * 512, (g + 1) * 512)
        wblocks = []
        for sec in range(3):  # q, k, v
            wb = np.concatenate(
                [Wqkv[sec * C:(sec + 1) * C][rsl].T,
                 Wdt[sec * C:(sec + 1) * C][rsl].T], axis=0)  # [2048, 512]
            wblocks.append(wb.reshape(KC, 128, 512))
        # w layout: [128, ob, kc, 512]
        w_r = np.ascontiguousarray(
            np.stack(wblocks, 0).transpose(2, 0, 1, 3).reshape(128, -1)
        ).astype(NPBF)
        xdt = np.concatenate([x[b].T, delta_t_emb[b].T], 0)  # [2048, N]
        xdt = np.ascontiguousarray(
            xdt.reshape(KC, 128, NT, 128).transpose(1, 2, 0, 3)
            .reshape(128, -1)).astype(NPBF)
        m = {
            "xdT": xdt,
            "w": w_r,
            "vres": pmaj((l2 * v_residual_v1[b, g * 8:(g + 1) * 8])
                         .transpose(1, 0, 2).reshape(N, 512)),
            "wproj": pmaj(Wproj[:, rsl].T),
            "rope": rope_r,
        }
        in_maps.append(m)

    trace = bool(int(os.environ.get("KERNEL_TRACE", "0")))
    res = run_bass_kernel_spmd(nc, in_maps, core_ids=list(range(8)),
                               trace=trace)
    if trace and res.exec_time_ns is not None:
        print(f"HW exec time: {res.exec_time_ns} ns")
        kernel.last_exec_time_ns = res.exec_time_ns
        kernel.last_results = res

    out = np.empty((B, N, C), np.float32)
    for b in range(B):
        out[b] = (res.results[2 * b]["out"].astype(np.float32)
                  + res.results[2 * b + 1]["out"].astype(np.float32))
    if np.any(bproj):
        out += bproj[None, None, :]
    return out


# revision 25
# speedup vs baseline: 1.2490x; 1.2490x over previous
"""Trainium2 Bass kernel for nn_AttentionBlock (B=4, N=1024, C=1024, H=16).

Sharding: 8 cores = 4 batches x 2 head-groups (8 heads each). Each core
computes its batch's tokens for its 8 heads end-to-end; the host sums the
two partial output projections per batch.

v2 design notes (vs the fp32r baseline):
- All matmul operands are bf16 (PSUM accumulation stays fp32). Inputs are
  converted host-side; DMA traffic and DVE element counts halve.
- Loop nests keep TensorE dense so the HAM clock-gate stays at 2.4 GHz.
- qk-LayerNorm stats via one bn_stats pass + per-head bn_aggr; rstd is
  exp(-0.5*ln(var+eps)) so ScalarE only ever uses the natural_log_exp
  table set (no ACT table reloads; softmax exp shares the same set).
- v tiles carry 64 ones-columns per head, so the attention a@v matmul
  also produces the softmax denominator replicated across the unused 64
  output partitions. Even heads store [v|ones], odd heads [ones|v], which
  lane-aligns values with outT rows for the normalize multiply.
- Softmax normalize: DVE reciprocal of one denominator row + GpSimd
  partition_broadcast + DVE multiply (no DRAM round-trip).
- exp is batched [128, 2x512] across the two heads of a pair (one ACT op
  per key tile).
"""
import os
import sys

sys.path.insert(0, "/opt/trn_rl_repo")

import numpy as np
import ml_dtypes

import concourse.bass as bass
import concourse.bacc as bacc
import concourse.tile as tile
from concourse import mybir
from concourse.bass_utils import run_bass_kernel_spmd
from concourse.masks import make_identity

F32 = mybir.dt.float32
BF16 = mybir.dt.bfloat16
NPBF = ml_dtypes.bfloat16

B, N, C, H = 4, 1024, 1024, 16
DH = C // H            # 64
HPC = 8                # heads per core
NT = N // 128          # 8 token tiles
KC = (2 * C) // 128    # 16 contraction chunks for fused qkv+dt
HD = DH // 2           # 32 (rope half)
EPS = 1e-5
ALU = mybir.AluOpType
AF = mybir.ActivationFunctionType
AXX = mybir.AxisListType.X


def _bcast_free(ap, n, axis_pos=1):
    """Insert a step-0 free dim of size n at axis_pos of an AP."""
    new = list(ap.ap)
    new.insert(axis_pos, [0, n])
    return bass.AP(tensor=ap.tensor, offset=ap.offset, ap=new)


def _bcast_part(ap, n):
    """Partition-broadcast AP (step-0 partition dim) for DMA use."""
    return bass.AP(tensor=ap.tensor, offset=ap.offset,
                   ap=[[0, n]] + list(ap.ap[1:]))


def build(l1):
    """Single-core SPMD program. l1: python float (lamb1). lamb2 is folded
    into vres host-side. Zero qkv/dt biases and identity qk-LN assumed
    (verified by kernel(); otherwise a numpy fallback runs)."""
    nc = bacc.Bacc("TRN2", target_bir_lowering=False)

    # All inputs pre-arranged host-side to [128, X] partition-major
    # contiguous layouts so each is one full-bandwidth DMA.
    xdT = nc.dram_tensor("xdT", [128, KC * N], BF16, kind="ExternalInput")
    w = nc.dram_tensor("w", [128, 3 * KC * 512], BF16, kind="ExternalInput")
    vres = nc.dram_tensor("vres", [128, NT * 512], BF16,
                          kind="ExternalInput")
    wproj = nc.dram_tensor("wproj", [128, 4 * C], BF16, kind="ExternalInput")
    rope = nc.dram_tensor("rope", [128, NT * 2 * DH], BF16,
                          kind="ExternalInput")
    outd = nc.dram_tensor("out", [N, C], BF16, kind="ExternalOutput")

    with tile.TileContext(nc) as tc:
        with (
            tc.tile_pool(name="const", bufs=1) as constp,
            tc.tile_pool(name="longp", bufs=1) as longp,
        ):
            ident = constp.tile([128, 128], BF16)
            make_identity(nc, ident)
            eps_t = constp.tile([128, 1], F32)
            nc.vector.memset(eps_t, EPS)

            xdT_sb = longp.tile([128, NT, KC, 128], BF16)
            w_sb = longp.tile([128, 3, KC, 512], BF16)
            rp_sb = longp.tile([128, NT, 2 * DH], BF16)
            vres_sb = longp.tile([128, NT, 512], BF16)
            wproj_sb = longp.tile([128, 4, C], BF16)
            v_sb = longp.tile([128, NT, HPC, 128], BF16)
            qT_sb = longp.tile([128, 4, N], BF16)
            kT_sb = longp.tile([128, 4, N], BF16)
            outT_sb = longp.tile([128, 4, N], BF16)
            # staging for the two-pass LN: raw q/k projections; the rope
            # output is written back IN PLACE (the raw values are dead by
            # then), so this doubles as the pre-transpose q/k buffer.
            stage_sb = longp.tile([128, 2, NT, 512], BF16)
            sums_sb = longp.tile([128, 2, NT, HPC], F32)
            sqs_sb = longp.tile([128, 2, NT, HPC], F32)
            rstd_sb = longp.tile([128, 2, NT, HPC], F32)
            nmr_sb = longp.tile([128, 2, NT, HPC], F32)

            # ones columns in v tiles: a@v against them replicates the
            # softmax denominator on the same lanes in a second PSUM bank.
            nc.gpsimd.memset(v_sb[:, :, :, 64:128], 1.0)

            # ---- input DMAs (contiguous, ordered by first use) ----
            nc.sync.dma_start(out=rp_sb, in_=rope[:, :])
            for c4 in range(4):  # k weights first, 4-kc chunks
                nc.sync.dma_start(
                    out=w_sb[:, 1, 4 * c4:4 * c4 + 4, :],
                    in_=w[:, (KC + 4 * c4) * 512:(KC + 4 * c4 + 4) * 512])
            for t in range(NT):  # x/dt in token-tile-major chunks
                nc.sync.dma_start(
                    out=xdT_sb[:, t, :, :],
                    in_=xdT[:, t * KC * 128:(t + 1) * KC * 128])
            nc.sync.dma_start(out=w_sb[:, 0, :, :], in_=w[:, 0:KC * 512])
            nc.sync.dma_start(out=w_sb[:, 2, :, :],
                              in_=w[:, 2 * KC * 512:3 * KC * 512])
            nc.sync.dma_start(out=vres_sb, in_=vres[:, :])
            nc.sync.dma_start(out=wproj_sb, in_=wproj[:, :])

            with (
                tc.tile_pool(name="psA", bufs=2, space="PSUM") as psA,
                tc.tile_pool(name="scr", bufs=2) as scr,
                tc.tile_pool(name="sml", bufs=2) as sml,
                tc.tile_pool(name="psS", bufs=2, space="PSUM") as psS,
                tc.tile_pool(name="psV", bufs=2, space="PSUM") as psV,
                tc.tile_pool(name="expp", bufs=11) as expp,
                tc.tile_pool(name="nrmp", bufs=2) as nrmp,
                tc.tile_pool(name="outp", bufs=2) as outp,
            ):
                def mm_tile(ob, t):
                    """16 accumulating matmuls for one (ob, token-tile)."""
                    ps = psA.tile([128, 512], F32, tag="ps")
                    for kc in range(KC):
                        nc.tensor.matmul(
                            ps[:],
                            xdT_sb[:, t, kc, :],
                            w_sb[:, ob, kc, :],
                            start=(kc == 0), stop=(kc == KC - 1))
                    return ps

                def a1_post(ob, t, ps):
                    """Stage the raw projection + accumulate LN stats."""
                    st3 = stage_sb[:, ob, t, :].rearrange(
                        "p (h d) -> p h d", h=HPC)
                    nc.scalar.copy(out=st3,
                                   in_=ps.rearrange("p (h d) -> p h d",
                                                    h=HPC))
                    nc.vector.reduce_sum(out=sums_sb[:, ob, t, :], in_=st3,
                                         axis=AXX)
                    sqt = scr.tile([128, HPC, DH], BF16, tag="sqt")
                    nc.gpsimd.tensor_tensor(out=sqt[:], in0=st3, in1=st3,
                                            op=ALU.mult)
                    nc.vector.reduce_sum(out=sqs_sb[:, ob, t, :], in_=sqt[:],
                                         axis=AXX)

                def stats_batch(ob, ts=slice(0, NT)):
                    """rstd/-mean*rstd for a token-tile range of one ob in
                    two ACT ops (avoids per-tile Ln/Exp table thrash)."""
                    nts = ts.stop - ts.start
                    mean = sml.tile([128, NT, HPC], F32, tag="mean")
                    mean = mean[:, 0:nts, :]
                    nc.vector.tensor_scalar_mul(mean,
                                                in0=sums_sb[:, ob, ts, :],
                                                scalar1=1.0 / DH)
                    msq = sml.tile([128, NT, HPC], F32, tag="msq")
                    msq = msq[:, 0:nts, :]
                    nc.vector.tensor_tensor(out=msq, in0=mean,
                                            in1=mean, op=ALU.mult)
                    var = sml.tile([128, NT, HPC], F32, tag="var")
                    var = var[:, 0:nts, :]
                    nc.vector.scalar_tensor_tensor(
                        out=var, in0=sqs_sb[:, ob, ts, :], scalar=1.0 / DH,
                        in1=msq, op0=ALU.mult, op1=ALU.subtract)
                    lv = sml.tile([128, NT, HPC], F32, tag="lv")
                    lv = lv[:, 0:nts, :]
                    nc.scalar.activation(out=lv, in_=var, func=AF.Ln,
                                         bias=eps_t[:])
                    nc.scalar.activation(out=rstd_sb[:, ob, ts, :], in_=lv,
                                         func=AF.Exp, scale=-0.5)
                    nc.vector.scalar_tensor_tensor(
                        out=nmr_sb[:, ob, ts, :], in0=mean, scalar=-1.0,
                        in1=rstd_sb[:, ob, ts, :], op0=ALU.mult, op1=ALU.mult)

                def a2_tile(ob, t):
                    """LN apply + rope, writing back into stage_sb."""
                    st3 = stage_sb[:, ob, t, :].rearrange(
                        "p (h d) -> p h d", h=HPC)
                    tmp = scr.tile([128, HPC, DH], BF16, tag="tmp")
                    nc.vector.tensor_tensor(
                        out=tmp[:], in0=st3,
                        in1=_bcast_free(rstd_sb[:, ob, t, :], DH, 2),
                        op=ALU.mult)
                    ln = scr.tile([128, HPC, DH], BF16, tag="ln")
                    nc.vector.tensor_tensor(
                        out=ln[:], in0=tmp[:],
                        in1=_bcast_free(nmr_sb[:, ob, t, :], DH, 2),
                        op=ALU.add)
                    sin0 = _bcast_free(rp_sb[:, t, 0:HD], HPC, 1)
                    sin1 = _bcast_free(rp_sb[:, t, HD:DH], HPC, 1)
                    cos0 = _bcast_free(rp_sb[:, t, DH:DH + HD], HPC, 1)
                    cos1 = _bcast_free(rp_sb[:, t, DH + HD:2 * DH], HPC, 1)
                    t1 = scr.tile([128, HPC, HD], BF16, tag="t1")
                    t2 = scr.tile([128, HPC, HD], BF16, tag="t2")
                    t3 = scr.tile([128, HPC, HD], BF16, tag="t3")
                    t4 = scr.tile([128, HPC, HD], BF16, tag="t4")
                    lo = ln[:, :, 0:HD]
                    hi = ln[:, :, HD:DH]
                    nc.gpsimd.tensor_tensor(out=t1[:], in0=hi, in1=sin0[:],
                                            op=ALU.mult)
                    nc.vector.tensor_tensor(out=t2[:], in0=lo, in1=cos0[:],
                                            op=ALU.mult)
                    nc.vector.tensor_tensor(out=st3[:, :, 0:HD],
                                            in0=t2[:], in1=t1[:],
                                            op=ALU.subtract)
                    nc.gpsimd.tensor_tensor(out=t3[:], in0=lo, in1=sin1[:],
                                            op=ALU.mult)
                    nc.vector.tensor_tensor(out=t4[:], in0=hi, in1=cos1[:],
                                            op=ALU.mult)
                    nc.vector.tensor_tensor(out=st3[:, :, HD:DH],
                                            in0=t4[:], in1=t3[:],
                                            op=ALU.add)

                def transposes(ob, dstT, th, cpeng=None):
                    cpeng = cpeng or nc.scalar
                    for j in range(4):
                        pt = psS.tile([128, 512], BF16, tag="sc")
                        for i in range(4):
                            t = th * 4 + i
                            nc.tensor.transpose(
                                pt[:, i * 128:(i + 1) * 128],
                                stage_sb[:, ob, t,
                                         j * 128:(j + 1) * 128],
                                ident[:])
                        if cpeng is nc.scalar:
                            nc.scalar.copy(
                                out=dstT[:, j, th * 512:(th + 1) * 512],
                                in_=pt[:])
                        else:
                            nc.vector.tensor_copy(
                                dstT[:, j, th * 512:(th + 1) * 512], pt[:])

                def sc_kc(j, qh, kc, exs):
                    """One score pair + exp for (j, qh) at key tile kc."""
                    qsl = slice(qh * 512, (qh + 1) * 512)
                    ksl = slice(kc * 128, (kc + 1) * 128)
                    sct = psS.tile([128, 2, 512], F32, tag="sc")
                    nc.tensor.matmul(
                        sct[:, 0, :], kT_sb[0:64, j, ksl],
                        qT_sb[0:64, j, qsl],
                        start=True, stop=True, tile_position=(0, 0))
                    nc.tensor.matmul(
                        sct[:, 1, :], kT_sb[64:128, j, ksl],
                        qT_sb[64:128, j, qsl],
                        start=True, stop=True, tile_position=(64, 0))
                    ex = expp.tile([128, 2, 512], BF16, tag="ex")
                    nc.scalar.activation(out=ex[:], in_=sct[:],
                                         func=AF.Exp, scale=0.125)
                    exs.append(ex)

                def sc_exp(j, qh, exs):
                    for kc in range(NT):
                        sc_kc(j, qh, kc, exs)

                def av_kc(st8, kc):
                    """a@v + denominator matmuls for one key tile of the
                    LAGGING head pair (software pipeline stage 2)."""
                    j, qh, exs, avv, avd = st8
                    ex = exs[kc]
                    st = (kc == 0)
                    sp = (kc == NT - 1)
                    nc.tensor.matmul(
                        avv[0:64, :], v_sb[:, kc, 2 * j, 0:64],
                        ex[:, 0, :], start=st, stop=sp,
                        tile_position=(0, 0))
                    nc.tensor.matmul(
                        avd[0:64, :], v_sb[:, kc, 2 * j, 64:128],
                        ex[:, 0, :], start=st, stop=sp,
                        tile_position=(0, 0))
                    nc.tensor.matmul(
                        avv[64:128, :], v_sb[:, kc, 2 * j + 1, 0:64],
                        ex[:, 1, :], start=st, stop=sp,
                        tile_position=(0, 64))
                    nc.tensor.matmul(
                        avd[64:128, :], v_sb[:, kc, 2 * j + 1, 64:128],
                        ex[:, 1, :], start=st, stop=sp,
                        tile_position=(0, 64))

                def norm(st8):
                    """Normalize the lagging pair: copy to SBUF (frees the
                    PSUM banks), reciprocal + multiply on DVE."""
                    j, qh, exs, avv, avd = st8
                    qsl = slice(qh * 512, (qh + 1) * 512)
                    exs.clear()
                    va = nrmp.tile([128, 512], F32, tag="va")
                    nc.vector.tensor_copy(va[:], avv[:])
                    da = nrmp.tile([128, 512], F32, tag="da")
                    nc.vector.tensor_copy(da[:], avd[:])
                    rcb = nrmp.tile([128, 512], F32, tag="rcb")
                    nc.vector.reciprocal_approx_fast(out=rcb[:], in_=da[:])
                    nc.vector.tensor_tensor(
                        out=outT_sb[:, j, qsl], in0=va[:], in1=rcb[:],
                        op=ALU.mult)

                def proj_tile(t, tail=False):
                    ost = outp.tile([128, C], BF16, tag="ost")
                    for oh in range(2):
                        pp = psA.tile([128, 512], F32, tag="ps")
                        for cc in range(4):
                            nc.tensor.matmul(
                                pp[:],
                                outT_sb[:, cc, t * 128:(t + 1) * 128],
                                wproj_sb[:, cc, oh * 512:(oh + 1) * 512],
                                start=(cc == 0), stop=(cc == 3))
                        if tail and oh == 1:
                            nc.scalar.copy(out=ost[:, 512:1024], in_=pp[:])
                        else:
                            nc.vector.tensor_copy(
                                ost[:, oh * 512:(oh + 1) * 512], pp[:])
                    nc.sync.dma_start(
                        out=outd[t * 128:(t + 1) * 128, :], in_=ost[:])

                def v_tile(t):
                    ps = mm_tile(2, t)
                    ps3 = ps.rearrange("p (h d) -> p h d", h=HPC)
                    vt3 = vres_sb[:, t, :].rearrange("p (h d) -> p h d",
                                                     h=HPC)
                    nc.vector.scalar_tensor_tensor(
                        out=v_sb[:, t, :, 0:64], in0=ps3, scalar=l1,
                        in1=vt3, op0=ALU.mult, op1=ALU.add)

                # ---- phase A (ordered to start the exp stream early) ----
                for t in range(NT):
                    a1_post(1, t, mm_tile(1, t))        # k
                stats_batch(1)
                for t in range(4):                      # v tiles 0-3
                    v_tile(t)
                    a2_tile(1, t)                       # k LN+rope rides
                for t in range(NT):                     # q
                    a1_post(0, t, mm_tile(0, t))
                    if t < 4:
                        a2_tile(1, t + 4)
                    else:
                        if t == 4:
                            stats_batch(0, slice(0, 4))
                        a2_tile(0, t - 4)
                transposes(1, kT_sb, 0)
                transposes(1, kT_sb, 1)
                transposes(0, qT_sb, 0)
                exs0 = []
                sc_exp(0, 0, exs0)                      # exp stream starts
                stats_batch(0, slice(4, NT))
                for t in range(4, NT):                  # v tiles 4-7
                    v_tile(t)
                    a2_tile(0, t)
                transposes(0, qT_sb, 1, cpeng=nc.vector)

                # ---- attention + projection (av lags one pair) ----
                avv0 = psV.tile([128, 512], F32, tag="av")
                avd0 = psV.tile([128, 512], F32, tag="av")
                prev = (0, 0, exs0, avv0, avd0)
                projq = []
                seq = [(j, qh) for qh in range(2) for j in range(4)]
                for j, qh in seq[1:]:
                    exs = []
                    for kc in range(NT):
                        sc_kc(j, qh, kc, exs)
                        av_kc(prev, kc)
                    norm(prev)
                    if prev[0] == 3 and prev[1] == 0:
                        projq = [0, 1, 2, 3]
                    if projq:
                        proj_tile(projq.pop(0))
                    avvn = psV.tile([128, 512], F32, name=f"avv{j}{qh}",
                                    tag="av")
                    avdn = psV.tile([128, 512], F32, name=f"avd{j}{qh}",
                                    tag="av")
                    prev = (j, qh, exs, avvn, avdn)
                for kc in range(NT):
                    av_kc(prev, kc)
                norm(prev)
                for t in projq + [4, 5, 6, 7]:
                    proj_tile(t, tail=True)

    nc.finalize()
    return nc


def _np_reference(x, rope, delta_t_emb, v_residual_v1, Wqkv, bqkv, Wdt, bdt,
                  qn_g, qn_b, kn_g, kn_b, lamb1, lamb2, Wproj, bproj):
    """Plain numpy fallback for input regimes the device kernel doesn't
    specialize (nonzero qkv/dt bias or non-identity qk-LN affine)."""
    b, n, c = x.shape
    qkv = (x @ Wqkv.T + bqkv).reshape(b, n, 3, H, DH).transpose(2, 0, 3, 1, 4)
    dqkv = (delta_t_emb @ Wdt.T + bdt).reshape(b, n, 3, H, DH)
    dqkv = dqkv.transpose(2, 0, 3, 1, 4)
    q = qkv[0] + dqkv[0]
    k = qkv[1] + dqkv[1]
    v = lamb1 * (qkv[2] + dqkv[2]) + lamb2 * v_residual_v1

    def ln(t, g, bb):
        m = t.mean(-1, keepdims=True)
        va = ((t - m) ** 2).mean(-1, keepdims=True)
        return (t - m) / np.sqrt(va + EPS) * g + bb

    q = ln(q, qn_g, qn_b)
    k = ln(k, kn_g, kn_b)
    sin, cos = rope[:, :DH], rope[:, DH:]

    def rot(t):
        h2 = np.concatenate([-t[..., DH // 2:], t[..., :DH // 2]], -1)
        return t * cos + h2 * sin

    q = rot(q)
    k = rot(k)
    s = np.einsum('bhqd,bhkd->bhqk', q, k) / np.sqrt(DH)
    s = s - s.max(-1, keepdims=True)
    e = np.exp(s)
    a = e / e.sum(-1, keepdims=True)
    o = np.einsum('bhqk,bhkd->bhqd', a, v)
    o = o.transpose(0, 2, 1, 3).reshape(b, n, c)
    return o @ Wproj.T + bproj


_CACHE = {}


def kernel(x, rope, delta_t_emb, v_residual_v1, Wqkv, bqkv, Wdt, bdt,
           qn_g, qn_b, kn_g, kn_b, lamb1, lamb2, Wproj, bproj):
    x = np.asarray(x, np.float32)
    rope = np.asarray(rope, np.float32)
    delta_t_emb = np.asarray(delta_t_emb, np.float32)
    v_residual_v1 = np.asarray(v_residual_v1, np.float32)
    Wqkv = np.asarray(Wqkv, np.float32)
    Wdt = np.asarray(Wdt, np.float32)
    Wproj = np.asarray(Wproj, np.float32)
    bqkv = np.asarray(bqkv, np.float32)
    bdt = np.asarray(bdt, np.float32)
    bproj = np.asarray(bproj, np.float32)
    qn_g = np.asarray(qn_g, np.float32)
    qn_b = np.asarray(qn_b, np.float32)
    kn_g = np.asarray(kn_g, np.float32)
    kn_b = np.asarray(kn_b, np.float32)
    l1 = float(np.asarray(lamb1))
    l2 = float(np.asarray(lamb2))

    general = (np.any(bqkv) or np.any(bdt) or np.any(qn_g != 1.0)
               or np.any(qn_b) or np.any(kn_g != 1.0) or np.any(kn_b))
    if general:
        return _np_reference(
            x, rope, delta_t_emb, v_residual_v1, Wqkv, bqkv, Wdt, bdt,
            qn_g, qn_b, kn_g, kn_b, l1, l2, Wproj, bproj).astype(np.float32)

    if l1 not in _CACHE:
        _CACHE[l1] = build(l1)
    nc = _CACHE[l1]

    def pmaj(a):
        """[G*128, X] -> [128, G*X] partition-major contiguous bf16."""
        g = a.shape[0] // 128
        return np.ascontiguousarray(
            a.reshape(g, 128, -1).transpose(1, 0, 2).reshape(128, -1)
        ).astype(NPBF)

    rope_r = pmaj(rope)
    in_maps = []
    for core in range(8):
        b = core // 2
        g = core % 2
        rsl = slice(g * 512, (g + 1) * 512)
        wblocks = []
        for sec in range(3):  # q, k, v
            wb = np.concatenate(
                [Wqkv[sec * C:(sec + 1) * C][rsl].T,
                 Wdt[sec * C:(sec + 1) * C][rsl].T], axis=0)  # [2048, 512]
            wblocks.append(wb.reshape(KC, 128, 512))
        # w layout: [128, ob, kc, 512]
        w_r = np.ascontiguousarray(
            np.stack(wblocks, 0).transpose(2, 0, 1, 3).reshape(128, -1)
        ).astype(NPBF)
        xdt = np.concatenate([x[b].T, delta_t_emb[b].T], 0)  # [2048, N]
        xdt = np.ascontiguousarray(
            xdt.reshape(KC, 128, NT, 128).transpose(1, 2, 0, 3)
            .reshape(128, -1)).astype(NPBF)
        m = {
            "xdT": xdt,
            "w": w_r,
            "vres": pmaj((l2 * v_residual_v1[b, g * 8:(g + 1) * 8])
                         .transpose(1, 0, 2).reshape(N, 512)),
            "wproj": pmaj(Wproj[:, rsl].T),
            "rope": rope_r,
        }
        in_maps.append(m)

    trace = bool(int(os.environ.get("KERNEL_TRACE", "0")))
    res = run_bass_kernel_spmd(nc, in_maps, core_ids=list(range(8)),
                               trace=trace)
    if trace and res.exec_time_ns is not None:
        print(f"HW exec time: {res.exec_time_ns} ns")
        kernel.last_exec_time_ns = res.exec_time_ns
        kernel.last_results = res

    out = np.empty((B, N, C), np.float32)
    for b in range(B):
        out[b] = (res.results[2 * b]["out"].astype(np.float32)
                  + res.results[2 * b + 1]["out"].astype(np.float32))
    if np.any(bproj):
        out += bproj[None, None, :]
    return out
